# revision 22
# baseline (speedup 1.0000x reference)
"""Trainium2 Bass kernel for nn_BBoxGenerator (segment_reduce).

mask_fg (256, 1, 512, 512) f32 -> boxes (256, 4) f32 [x0, y0, x1, y1].

Pure data parallel: each of the 8 cores handles 32 images independently.

v8: HWDGE (nc.sync) f32 stream + reworked compute/finishing.
  Why HWDGE: SWDGE descriptor generation runs on GpSimd's Q7 and its
  descriptor rings live in SBUF; DVE tensor_scalar ops (our per-image
  threshold) enter 2-port perf mode which blocks GpSimd, and the ring
  fetches contend on the AXI ports serving SDMA engines 7/15 - traced
  as a sustained +16% per-descriptor slowdown on DMA_15 that paced the
  whole stream (v3-v7, 120-126us). HWDGE has no SBUF rings and never
  contends with DVE; the cost is losing the f32->bf16 cast-during-DMA,
  so images land as f32 (16 x 1 MiB SBUF buffers) and the threshold
  ops read f32.
  - Unified "positive iff any" masks: DVE rows use (x > 0.5) {0,1} with
    fused accum row sums; ACT rows use Relu(x-0.5) >= 0 with fused accum
    row sums. Row sums and PSUM column sums are all "> 0.002 iff any
    foreground": one threshold everywhere, PSUM rows 0..31 in image
    order, ONE output DMA (no un-permute).
  - Row side: per-group masked-iota min/max into rvals during the
    stream; one PE transpose + two small reduces at the tail.
  - Col side: fused (colsum > thr) * iota via scalar_tensor_tensor +
    tensor_reduce min/max. (tensor_tensor_reduce would fuse further but
    crashes the HW: NRT_EXEC_UNIT_UNRECOVERABLE, micro_test.py stage 5.)
  - Box expand fused without predicates:
    lo' = min(lo, max(0, c - s/2)), hi' = max(hi, min(1, c + s/2)).
  - Q7 (gpsimd) paces SWDGE descriptor emission ring-full, so const
    emission is split: tiny block (one-hot + row tables) right after
    dma1 while the ring is shallow; tail-only consts before dma30 where
    the deep ring absorbs the pause.
"""

import numpy as np

from concourse import bacc, mybir
from concourse.tile import TileContext
from concourse.bass_utils import run_bass_kernel_spmd

F32 = mybir.dt.float32
BF16 = mybir.dt.bfloat16
I32 = mybir.dt.int32
OP = mybir.AluOpType
AX = mybir.AxisListType
AF = mybir.ActivationFunctionType

N_CORES = 8
B = 256
BP = B // N_CORES  # 32 images per core
H = W = 512
IMG_FREE = 4 * W  # 2048 free elems per image (4 rows per partition)
GROUP = 8
OHW = 64  # one-hot block width per image (v1 width; rows 32..63 unused)

MIN_BOX = 0.05
ANY_THR = 0.002  # any_t/PSUM sums exceed this iff any foreground


def build_nc():
    nc = bacc.Bacc("TRN2", target_bir_lowering=False, debug=False, num_devices=N_CORES)
    x = nc.declare_dram_parameter("mask_fg", [BP, 1, H, W], F32, isOutput=False)
    out = nc.declare_dram_parameter("out", [BP, 4], F32, isOutput=True)

    # (128, BP, 4, 512): partition p holds rows 4p..4p+3 of each image
    xv = x.ap().rearrange("b one (p a) w -> p (b one) a w", p=128)

    with TileContext(nc) as tc:
        with (
            tc.tile_pool(name="consts", bufs=1) as consts,
            tc.tile_pool(name="imgs", bufs=16) as imgs,
            tc.tile_pool(name="masks", bufs=6) as masks,
            tc.tile_pool(name="small", bufs=1) as small,
            tc.tile_pool(name="pcol", bufs=1, space="PSUM") as pcol_pool,
            tc.tile_pool(name="ptr", bufs=1, space="PSUM") as ptr_pool,
        ):
            psum_col = pcol_pool.tile([OHW, W], F32)
            oh = consts.tile([128, BP * OHW], BF16)
            ones_oh = consts.tile([128, BP * OHW], BF16)
            any_t = small.tile([128, 4 * BP], F32)
            rvals = small.tile([128, 2 * BP], F32)

            neg_half = consts.tile([128, 1], F32)
            hm_lo_i = consts.tile([128, GROUP * 4], I32)
            hm_lo = consts.tile([128, GROUP * 4], F32)
            hm_hi_i = consts.tile([128, GROUP * 4], I32)
            hm_hi = consts.tile([128, GROUP * 4], F32)
            wm_lo_i = consts.tile([BP, W], I32)
            wm_lo = consts.tile([BP, W], F32)
            wm_hi_i = consts.tile([BP, W], I32)
            wm_hi = consts.tile([BP, W], F32)
            ones128 = consts.tile([128, 128], F32)
            ident = consts.tile([128, 128], F32)
            dflt = consts.tile([BP, 4], F32)

            def emit_early_consts():
                nc.gpsimd.memset(neg_half[:], -0.5)
                # OH[p, i*OHW + i] = 1: routes image i to PSUM row i
                nc.gpsimd.memset(ones_oh[:], 1.0)
                nc.gpsimd.affine_select(
                    oh[:], ones_oh[:], [[-1, BP], [1, OHW]], OP.is_equal, 0.0,
                    base=0, channel_multiplier=0,
                )
                # row-index tables: y = 4p + r; lo = y - 512, hi = y + 1
                nc.gpsimd.iota(hm_lo_i[:], [[0, GROUP], [1, 4]], base=-512,
                               channel_multiplier=4)
                nc.gpsimd.tensor_copy(hm_lo[:], hm_lo_i[:])
                nc.gpsimd.iota(hm_hi_i[:], [[0, GROUP], [1, 4]], base=1,
                               channel_multiplier=4)
                nc.gpsimd.tensor_copy(hm_hi[:], hm_hi_i[:])

            def emit_tail_consts():
                # col-index tables: lo = j - 512, hi = j + 1
                nc.gpsimd.iota(wm_lo_i[:], [[1, W]], base=-512, channel_multiplier=0)
                nc.gpsimd.tensor_copy(wm_lo[:], wm_lo_i[:])
                nc.gpsimd.iota(wm_hi_i[:], [[1, W]], base=1, channel_multiplier=0)
                nc.gpsimd.tensor_copy(wm_hi[:], wm_hi_i[:])
                nc.gpsimd.memset(ones128[:], 1.0)
                nc.gpsimd.affine_select(
                    ident[:], ones128[:], [[-1, 128]], OP.is_equal, 0.0,
                    base=0, channel_multiplier=1,
                )
                nc.gpsimd.memset(dflt[:, 0:2], 0.25)
                nc.gpsimd.memset(dflt[:, 2:4], 0.75)

            # row-side groups; last group is a single image
            row_groups = [(0, 8), (8, 8), (16, 8), (24, 7), (31, 1)]

            def finish_group(start, n):
                cs = slice(4 * start, 4 * (start + n))
                rt_lo = small.tile([128, 4 * GROUP], F32, tag="rt_lo")
                nc.vector.scalar_tensor_tensor(
                    rt_lo[:, 0:4 * n], any_t[:, cs], ANY_THR, hm_lo[:, 0:4 * n],
                    OP.is_gt, OP.mult)
                nc.vector.tensor_reduce(
                    rvals[:, start:start + n],
                    rt_lo[:, 0:4 * n].rearrange("p (i r) -> p i r", r=4),
                    op=OP.min, axis=AX.X)
                rt_hi = small.tile([128, 4 * GROUP], F32, tag="rt_hi")
                nc.vector.scalar_tensor_tensor(
                    rt_hi[:, 0:4 * n], any_t[:, cs], ANY_THR, hm_hi[:, 0:4 * n],
                    OP.is_gt, OP.mult)
                nc.vector.tensor_reduce(
                    rvals[:, BP + start:BP + start + n],
                    rt_hi[:, 0:4 * n].rearrange("p (i r) -> p i r", r=4),
                    op=OP.max, axis=AX.X)

            # per-image row-block engine split: 'd' = DVE is_gt {0,1} count,
            # 'a' = ACT Relu(x-0.5) sum. For the last two images DVE takes the
            # late row blocks (it runs the rest of the tail anyway).
            def row_engines(i):
                if i >= BP - 2:
                    return "aadd"
                return "ddaa"

            # first image starts with a small piece so the stream begins
            # sooner; last two stream per row-block so tail threshold/matmul
            # latency tracks the last arriving bytes (HWDGE: pieces cost no
            # Q7 time)
            def dma_pieces(i):
                if i == 0:
                    return [(0, 1), (1, 4)]
                if i >= BP - 2:
                    return [(0, 1), (1, 2), (2, 3), (3, 4)]
                return [(0, 4)]

            def emit_dma(i):
                img = imgs.tile([128, IMG_FREE], F32, tag="img")
                img3 = img[:].rearrange("p (a w) -> p a w", a=4)
                for (al, ah) in dma_pieces(i):
                    nc.sync.dma_start(
                        out=img3[:, al:ah, :],
                        in_=xv[:, i:i + 1, al:ah],
                    )
                return img

            def emit_compute(i, img):
                m01 = masks.tile([128, IMG_FREE], BF16, tag="m01")
                for r, eng in enumerate(row_engines(i)):
                    sl = slice(r * W, (r + 1) * W)
                    acc = any_t[:, 4 * i + r:4 * i + r + 1]
                    if eng == "d":
                        nc.vector.tensor_scalar(
                            m01[:, sl], img[:, sl], 0.5, None,
                            OP.is_gt, OP.add, accum_out=acc)
                    else:
                        nc.scalar.activation(
                            m01[:, sl], img[:, sl], AF.Relu,
                            bias=neg_half[:], accum_out=acc)
                for r in range(4):
                    sl = slice(r * W, (r + 1) * W)
                    nc.tensor.matmul(
                        psum_col[:, :], oh[:, i * OHW:i * OHW + OHW], m01[:, sl],
                        start=(i == 0 and r == 0), stop=(i == BP - 1 and r == 3),
                    )
                for (gs, gn) in row_groups:
                    if gs + gn - 1 == i:
                        finish_group(gs, gn)

            emit_early_consts()
            emit_tail_consts()
            for i in range(BP):
                img = emit_dma(i)
                emit_compute(i, img)

            # ---- tail ----
            braw = small.tile([BP, 4], F32)

            # col side: (colsum > thr) * iota, masked min/max (rows 0..31)
            scr_lo = small.tile([BP, W], F32)
            scr_hi = small.tile([BP, W], F32)
            nc.vector.scalar_tensor_tensor(
                scr_lo[:], psum_col[0:BP, :], ANY_THR, wm_lo[:], OP.is_gt, OP.mult)
            nc.vector.tensor_reduce(braw[:, 0:1], scr_lo[:], op=OP.min, axis=AX.X)
            nc.vector.scalar_tensor_tensor(
                scr_hi[:], psum_col[0:BP, :], ANY_THR, wm_hi[:], OP.is_gt, OP.mult)
            nc.vector.tensor_reduce(braw[:, 2:3], scr_hi[:], op=OP.max, axis=AX.X)

            # row side: transpose rvals, reduce per image
            rT = ptr_pool.tile([2 * BP, 128], F32)
            nc.tensor.transpose(rT[:], rvals[:], ident[:])
            nc.vector.tensor_reduce(braw[:, 1:2], rT[0:BP, :], op=OP.min, axis=AX.X)
            nc.vector.tensor_reduce(braw[:, 3:4], rT[BP:2 * BP, :], op=OP.max, axis=AX.X)

            # empty mask: y_max raw is 0 iff no foreground
            emp = small.tile([BP, 1], F32)
            nc.vector.tensor_scalar(emp[:], braw[:, 3:4], 0.5, None, OP.is_lt)

            # normalize: lo = (v + 512)/512, hi = (v - 1)/512
            boxes = small.tile([BP, 4], F32)
            nc.vector.tensor_scalar(
                boxes[:, 0:2], braw[:, 0:2], 512.0, 1.0 / 512, OP.add, OP.mult)
            nc.vector.tensor_scalar(
                boxes[:, 2:4], braw[:, 2:4], 1.0, 1.0 / 512, OP.subtract, OP.mult)

            # expand-small fused: lo' = min(lo, max(0, c - s/2)),
            #                     hi' = max(hi, min(1, c + s/2))
            csum = small.tile([BP, 2], F32)
            lo2 = small.tile([BP, 2], F32)
            hi2 = small.tile([BP, 2], F32)
            final = small.tile([BP, 4], F32)
            nc.vector.tensor_add(csum[:], boxes[:, 0:2], boxes[:, 2:4])
            nc.vector.tensor_scalar(
                lo2[:], csum[:], 0.5, MIN_BOX * 0.5, OP.mult, OP.subtract)
            nc.vector.tensor_scalar(lo2[:], lo2[:], 0.0, None, OP.max)
            nc.vector.tensor_tensor(final[:, 0:2], boxes[:, 0:2], lo2[:], op=OP.min)
            nc.vector.tensor_scalar(
                hi2[:], csum[:], 0.5, MIN_BOX * 0.5, OP.mult, OP.add)
            nc.vector.tensor_scalar(hi2[:], hi2[:], 1.0, None, OP.min)
            nc.vector.tensor_tensor(final[:, 2:4], boxes[:, 2:4], hi2[:], op=OP.max)

            # default box where empty: final += (default - final) * emp
            dmb = small.tile([BP, 4], F32)
            nc.vector.tensor_sub(dmb[:], dflt[:], final[:])
            outb = small.tile([BP, 4], F32)
            nc.vector.scalar_tensor_tensor(
                outb[:], dmb[:], emp[:], final[:], OP.mult, OP.add)

            nc.sync.dma_start(out=out.ap(), in_=outb[:])

    return nc


_NC = None


def _get_nc():
    global _NC
    if _NC is None:
        nc = build_nc()
        nc.compile()
        _NC = nc
    return _NC


def kernel(mask_fg: np.ndarray) -> np.ndarray:
    mask_fg = np.ascontiguousarray(np.asarray(mask_fg, dtype=np.float32))
    assert mask_fg.shape == (B, 1, H, W), mask_fg.shape
    nc = _get_nc()
    shards = mask_fg.reshape(N_CORES, BP, 1, H, W)
    in_maps = [{"mask_fg": np.ascontiguousarray(shards[i])} for i in range(N_CORES)]
    res = run_bass_kernel_spmd(nc, in_maps, core_ids=list(range(N_CORES)))
    return np.concatenate(
        [res.results[i]["out"] for i in range(N_CORES)], axis=0
    ).astype(np.float32)


# revision 23
# speedup vs baseline: 1.0368x; 1.0368x over previous
"""Trainium2 Bass kernel for nn_BBoxGenerator (segment_reduce).

mask_fg (256, 1, 512, 512) f32 -> boxes (256, 4) f32 [x0, y0, x1, y1].

Pure data parallel: each of the 8 cores handles 32 images independently.

v8: HWDGE (nc.sync) f32 stream + reworked compute/finishing.
  Why HWDGE: SWDGE descriptor generation runs on GpSimd's Q7 and its
  descriptor rings live in SBUF; DVE tensor_scalar ops (our per-image
  threshold) enter 2-port perf mode which blocks GpSimd, and the ring
  fetches contend on the AXI ports serving SDMA engines 7/15 - traced
  as a sustained +16% per-descriptor slowdown on DMA_15 that paced the
  whole stream (v3-v7, 120-126us). HWDGE has no SBUF rings and never
  contends with DVE; the cost is losing the f32->bf16 cast-during-DMA,
  so images land as f32 (16 x 1 MiB SBUF buffers) and the threshold
  ops read f32.
  - Unified "positive iff any" masks: DVE rows use (x > 0.5) {0,1} with
    fused accum row sums; ACT rows use Relu(x-0.5) >= 0 with fused accum
    row sums. Row sums and PSUM column sums are all "> 0.002 iff any
    foreground": one threshold everywhere, PSUM rows 0..31 in image
    order, ONE output DMA (no un-permute).
  - Row side: per-group masked-iota min/max into rvals during the
    stream; one PE transpose + two small reduces at the tail.
  - Col side: fused (colsum > thr) * iota via scalar_tensor_tensor +
    tensor_reduce min/max. (tensor_tensor_reduce would fuse further but
    crashes the HW: NRT_EXEC_UNIT_UNRECOVERABLE, micro_test.py stage 5.)
  - Box expand fused without predicates:
    lo' = min(lo, max(0, c - s/2)), hi' = max(hi, min(1, c + s/2)).
  - Q7 (gpsimd) paces SWDGE descriptor emission ring-full, so const
    emission is split: tiny block (one-hot + row tables) right after
    dma1 while the ring is shallow; tail-only consts before dma30 where
    the deep ring absorbs the pause.
"""

import numpy as np

from concourse import bacc, mybir
from concourse.tile import TileContext
from concourse.bass_utils import run_bass_kernel_spmd

F32 = mybir.dt.float32
BF16 = mybir.dt.bfloat16
I32 = mybir.dt.int32
OP = mybir.AluOpType
AX = mybir.AxisListType
AF = mybir.ActivationFunctionType

N_CORES = 8
B = 256
BP = B // N_CORES  # 32 images per core
H = W = 512
IMG_FREE = 4 * W  # 2048 free elems per image (4 rows per partition)
GROUP = 8
OHW = 64  # one-hot block width per image (v1 width; rows 32..63 unused)

MIN_BOX = 0.05
ANY_THR = 0.002  # any_t/PSUM sums exceed this iff any foreground


def build_nc():
    nc = bacc.Bacc("TRN2", target_bir_lowering=False, debug=False, num_devices=N_CORES)
    x = nc.declare_dram_parameter("mask_fg", [BP, 1, H, W], F32, isOutput=False)
    out = nc.declare_dram_parameter("out", [BP, 4], F32, isOutput=True)

    # (128, BP, 4, 512): partition p holds rows 4p..4p+3 of each image
    xv = x.ap().rearrange("b one (p a) w -> p (b one) a w", p=128)

    with TileContext(nc) as tc:
        with (
            tc.tile_pool(name="consts", bufs=1) as consts,
            tc.tile_pool(name="imgs", bufs=16) as imgs,
            tc.tile_pool(name="masks", bufs=6) as masks,
            tc.tile_pool(name="small", bufs=1) as small,
            tc.tile_pool(name="pcol", bufs=1, space="PSUM") as pcol_pool,
            tc.tile_pool(name="ptr", bufs=1, space="PSUM") as ptr_pool,
        ):
            psum_col = pcol_pool.tile([OHW, W], F32)
            oh = consts.tile([128, BP * OHW], BF16)
            ones_oh = consts.tile([128, BP * OHW], BF16)
            any_t = small.tile([128, 4 * BP], F32)
            rvals = small.tile([128, 2 * BP], F32)

            neg_half = consts.tile([128, 1], F32)
            hm_lo_i = consts.tile([128, GROUP * 4], I32)
            hm_lo = consts.tile([128, GROUP * 4], F32)
            hm_hi_i = consts.tile([128, GROUP * 4], I32)
            hm_hi = consts.tile([128, GROUP * 4], F32)
            wm_lo_i = consts.tile([BP, W], I32)
            wm_lo = consts.tile([BP, W], BF16)
            wm_hi_i = consts.tile([BP, W], I32)
            wm_hi = consts.tile([BP, W], BF16)
            ones128 = consts.tile([128, 128], F32)
            ident = consts.tile([128, 128], F32)
            dflt = consts.tile([BP, 4], F32)

            def emit_early_consts():
                nc.gpsimd.memset(neg_half[:], -0.5)
                # OH[p, i*OHW + i] = 1: routes image i to PSUM row i
                nc.gpsimd.memset(ones_oh[:], 1.0)
                nc.gpsimd.affine_select(
                    oh[:], ones_oh[:], [[-1, BP], [1, OHW]], OP.is_equal, 0.0,
                    base=0, channel_multiplier=0,
                )
                # row-index tables: y = 4p + r; lo = y - 512, hi = y + 1
                nc.gpsimd.iota(hm_lo_i[:], [[0, GROUP], [1, 4]], base=-512,
                               channel_multiplier=4)
                nc.gpsimd.tensor_copy(hm_lo[:], hm_lo_i[:])
                nc.gpsimd.iota(hm_hi_i[:], [[0, GROUP], [1, 4]], base=1,
                               channel_multiplier=4)
                nc.gpsimd.tensor_copy(hm_hi[:], hm_hi_i[:])

            def emit_tail_consts():
                # col-index tables: lo = j - 512, hi = j + 1
                nc.gpsimd.iota(wm_lo_i[:], [[1, W]], base=-512, channel_multiplier=0)
                nc.gpsimd.tensor_copy(wm_lo[:], wm_lo_i[:])
                nc.gpsimd.iota(wm_hi_i[:], [[1, W]], base=1, channel_multiplier=0)
                nc.gpsimd.tensor_copy(wm_hi[:], wm_hi_i[:])
                nc.gpsimd.memset(ones128[:], 1.0)
                nc.gpsimd.affine_select(
                    ident[:], ones128[:], [[-1, 128]], OP.is_equal, 0.0,
                    base=0, channel_multiplier=1,
                )
                nc.gpsimd.memset(dflt[:, 0:2], 0.25)
                nc.gpsimd.memset(dflt[:, 2:4], 0.75)

            # row-side groups; last group is a single image
            row_groups = [(0, 8), (8, 8), (16, 8), (24, 7), (31, 1)]

            def finish_group(start, n):
                cs = slice(4 * start, 4 * (start + n))
                rt_lo = small.tile([128, 4 * GROUP], F32, tag="rt_lo")
                nc.vector.scalar_tensor_tensor(
                    rt_lo[:, 0:4 * n], any_t[:, cs], ANY_THR, hm_lo[:, 0:4 * n],
                    OP.is_gt, OP.mult)
                nc.vector.tensor_reduce(
                    rvals[:, start:start + n],
                    rt_lo[:, 0:4 * n].rearrange("p (i r) -> p i r", r=4),
                    op=OP.min, axis=AX.X)
                rt_hi = small.tile([128, 4 * GROUP], F32, tag="rt_hi")
                nc.vector.scalar_tensor_tensor(
                    rt_hi[:, 0:4 * n], any_t[:, cs], ANY_THR, hm_hi[:, 0:4 * n],
                    OP.is_gt, OP.mult)
                nc.vector.tensor_reduce(
                    rvals[:, BP + start:BP + start + n],
                    rt_hi[:, 0:4 * n].rearrange("p (i r) -> p i r", r=4),
                    op=OP.max, axis=AX.X)

            # per-image row-block engine split: 'd' = DVE is_gt {0,1} count,
            # 'a' = ACT Relu(x-0.5) sum. For the last two images DVE takes the
            # late row blocks (it runs the rest of the tail anyway).
            def row_engines(i):
                if i >= BP - 2:
                    return "aadd"
                return "ddaa"

            # first image starts with a small piece so the stream begins
            # sooner; last two stream per row-block so tail threshold/matmul
            # latency tracks the last arriving bytes (HWDGE: pieces cost no
            # Q7 time)
            def dma_pieces(i):
                if i == 0:
                    return [(0, 1), (1, 4)]
                return [(0, 4)]

            def emit_dma(i):
                img = imgs.tile([128, IMG_FREE], F32, tag="img")
                img3 = img[:].rearrange("p (a w) -> p a w", a=4)
                for (al, ah) in dma_pieces(i):
                    nc.sync.dma_start(
                        out=img3[:, al:ah, :],
                        in_=xv[:, i:i + 1, al:ah],
                    )
                return img

            def emit_compute(i, img):
                m01 = masks.tile([128, IMG_FREE], BF16, tag="m01")
                for r, eng in enumerate(row_engines(i)):
                    sl = slice(r * W, (r + 1) * W)
                    acc = any_t[:, 4 * i + r:4 * i + r + 1]
                    if eng == "d":
                        nc.vector.tensor_scalar(
                            m01[:, sl], img[:, sl], 0.5, None,
                            OP.is_gt, OP.add, accum_out=acc)
                    else:
                        nc.scalar.activation(
                            m01[:, sl], img[:, sl], AF.Relu,
                            bias=neg_half[:], accum_out=acc)
                for r in range(4):
                    sl = slice(r * W, (r + 1) * W)
                    nc.tensor.matmul(
                        psum_col[:, :], oh[:, i * OHW:i * OHW + OHW], m01[:, sl],
                        start=(i == 0 and r == 0), stop=(i == BP - 1 and r == 3),
                    )
                for (gs, gn) in row_groups:
                    if gs + gn - 1 == i:
                        finish_group(gs, gn)

            emit_early_consts()
            emit_tail_consts()
            for i in range(BP):
                img = emit_dma(i)
                emit_compute(i, img)

            # ---- tail ----
            braw = small.tile([BP, 4], F32)

            # col side: one (colsum > thr) any-mask off PSUM (f32 read), then
            # bf16 iota products + reduces (2x DVE rate; bf16 iota rounding is
            # <= 2 columns = 0.4% of W, well inside tolerance)
            colany = small.tile([BP, W], BF16)
            nc.vector.tensor_scalar(
                colany[:], psum_col[0:BP, :], ANY_THR, None, OP.is_gt)
            scr_lo = small.tile([BP, W], BF16)
            scr_hi = small.tile([BP, W], BF16)
            nc.vector.tensor_tensor(scr_lo[:], colany[:], wm_lo[:], op=OP.mult)
            nc.vector.tensor_reduce(braw[:, 0:1], scr_lo[:], op=OP.min, axis=AX.X)
            nc.vector.tensor_tensor(scr_hi[:], colany[:], wm_hi[:], op=OP.mult)
            nc.vector.tensor_reduce(braw[:, 2:3], scr_hi[:], op=OP.max, axis=AX.X)

            # row side: transpose rvals, reduce per image
            rT = ptr_pool.tile([2 * BP, 128], F32)
            nc.tensor.transpose(rT[:], rvals[:], ident[:])
            nc.vector.tensor_reduce(braw[:, 1:2], rT[0:BP, :], op=OP.min, axis=AX.X)
            nc.vector.tensor_reduce(braw[:, 3:4], rT[BP:2 * BP, :], op=OP.max, axis=AX.X)

            # empty mask: y_max raw is 0 iff no foreground
            emp = small.tile([BP, 1], F32)
            nc.vector.tensor_scalar(emp[:], braw[:, 3:4], 0.5, None, OP.is_lt)

            # normalize: lo = (v + 512)/512, hi = (v - 1)/512
            boxes = small.tile([BP, 4], F32)
            nc.vector.tensor_scalar(
                boxes[:, 0:2], braw[:, 0:2], 512.0, 1.0 / 512, OP.add, OP.mult)
            nc.vector.tensor_scalar(
                boxes[:, 2:4], braw[:, 2:4], 1.0, 1.0 / 512, OP.subtract, OP.mult)

            # expand-small fused: lo' = min(lo, max(0, c - s/2)),
            #                     hi' = max(hi, min(1, c + s/2))
            csum = small.tile([BP, 2], F32)
            lo2 = small.tile([BP, 2], F32)
            hi2 = small.tile([BP, 2], F32)
            final = small.tile([BP, 4], F32)
            nc.vector.tensor_add(csum[:], boxes[:, 0:2], boxes[:, 2:4])
            nc.vector.tensor_scalar(
                lo2[:], csum[:], 0.5, MIN_BOX * 0.5, OP.mult, OP.subtract)
            nc.vector.tensor_scalar(lo2[:], lo2[:], 0.0, None, OP.max)
            nc.vector.tensor_tensor(final[:, 0:2], boxes[:, 0:2], lo2[:], op=OP.min)
            nc.vector.tensor_scalar(
                hi2[:], csum[:], 0.5, MIN_BOX * 0.5, OP.mult, OP.add)
            nc.vector.tensor_scalar(hi2[:], hi2[:], 1.0, None, OP.min)
            nc.vector.tensor_tensor(final[:, 2:4], boxes[:, 2:4], hi2[:], op=OP.max)

            # default box where empty: final += (default - final) * emp
            dmb = small.tile([BP, 4], F32)
            nc.vector.tensor_sub(dmb[:], dflt[:], final[:])
            outb = small.tile([BP, 4], F32)
            nc.vector.scalar_tensor_tensor(
                outb[:], dmb[:], emp[:], final[:], OP.mult, OP.add)

            nc.sync.dma_start(out=out.ap(), in_=outb[:])

    return nc


_NC = None


def _get_nc():
    global _NC
    if _NC is None:
        nc = build_nc()
        nc.compile()
        _NC = nc
    return _NC


def kernel(mask_fg: np.ndarray) -> np.ndarray:
    mask_fg = np.ascontiguousarray(np.asarray(mask_fg, dtype=np.float32))
    assert mask_fg.shape == (B, 1, H, W), mask_fg.shape
    nc = _get_nc()
    shards = mask_fg.reshape(N_CORES, BP, 1, H, W)
    in_maps = [{"mask_fg": np.ascontiguousarray(shards[i])} for i in range(N_CORES)]
    res = run_bass_kernel_spmd(nc, in_maps, core_ids=list(range(N_CORES)))
    return np.concatenate(
        [res.results[i]["out"] for i in range(N_CORES)], axis=0
    ).astype(np.float32)


# revision 24
# speedup vs baseline: 1.1481x; 1.1073x over previous
"""Trainium2 Bass kernel for nn_BBoxGenerator (segment_reduce).

mask_fg (256, 1, 512, 512) f32 -> boxes (256, 4) f32 [x0, y0, x1, y1].

Pure data parallel: each of the 8 cores handles 32 images independently.

v8: HWDGE (nc.sync) f32 stream + reworked compute/finishing.
  Why HWDGE: SWDGE descriptor generation runs on GpSimd's Q7 and its
  descriptor rings live in SBUF; DVE tensor_scalar ops (our per-image
  threshold) enter 2-port perf mode which blocks GpSimd, and the ring
  fetches contend on the AXI ports serving SDMA engines 7/15 - traced
  as a sustained +16% per-descriptor slowdown on DMA_15 that paced the
  whole stream (v3-v7, 120-126us). HWDGE has no SBUF rings and never
  contends with DVE; the cost is losing the f32->bf16 cast-during-DMA,
  so images land as f32 (16 x 1 MiB SBUF buffers) and the threshold
  ops read f32.
  - Unified "positive iff any" masks: DVE rows use (x > 0.5) {0,1} with
    fused accum row sums; ACT rows use Relu(x-0.5) >= 0 with fused accum
    row sums. Row sums and PSUM column sums are all "> 0.002 iff any
    foreground": one threshold everywhere, PSUM rows 0..31 in image
    order, ONE output DMA (no un-permute).
  - Row side: per-group masked-iota min/max into rvals during the
    stream; one PE transpose + two small reduces at the tail.
  - Col side: fused (colsum > thr) * iota via scalar_tensor_tensor +
    tensor_reduce min/max. (tensor_tensor_reduce would fuse further but
    crashes the HW: NRT_EXEC_UNIT_UNRECOVERABLE, micro_test.py stage 5.)
  - Box expand fused without predicates:
    lo' = min(lo, max(0, c - s/2)), hi' = max(hi, min(1, c + s/2)).
  - Q7 (gpsimd) paces SWDGE descriptor emission ring-full, so const
    emission is split: tiny block (one-hot + row tables) right after
    dma1 while the ring is shallow; tail-only consts before dma30 where
    the deep ring absorbs the pause.
"""

import numpy as np

from concourse import bacc, mybir
from concourse.tile import TileContext
from concourse.bass_utils import run_bass_kernel_spmd

F32 = mybir.dt.float32
BF16 = mybir.dt.bfloat16
I32 = mybir.dt.int32
OP = mybir.AluOpType
AX = mybir.AxisListType
AF = mybir.ActivationFunctionType

N_CORES = 8
B = 256
BP = B // N_CORES  # 32 images per core
H = W = 512
IMG_FREE = 4 * W  # 2048 free elems per image (4 rows per partition)
GROUP = 8
OHW = 64  # one-hot block width per image (v1 width; rows 32..63 unused)

MIN_BOX = 0.05
ANY_THR = 0.002  # any_t/PSUM sums exceed this iff any foreground


def build_nc():
    nc = bacc.Bacc("TRN2", target_bir_lowering=False, debug=False, num_devices=N_CORES)
    x = nc.declare_dram_parameter("mask_fg", [BP, 1, H, W], F32, isOutput=False)
    out = nc.declare_dram_parameter("out", [BP, 4], F32, isOutput=True)

    # (128, BP, 4, 512): partition p holds rows 4p..4p+3 of each image
    xv = x.ap().rearrange("b one (p a) w -> p (b one) a w", p=128)

    with TileContext(nc) as tc:
        with (
            tc.tile_pool(name="consts", bufs=1) as consts,
            tc.tile_pool(name="imgs", bufs=16) as imgs,
            tc.tile_pool(name="masks", bufs=6) as masks,
            tc.tile_pool(name="small", bufs=1) as small,
            tc.tile_pool(name="pcol", bufs=1, space="PSUM") as pcol_pool,
            tc.tile_pool(name="ptr", bufs=1, space="PSUM") as ptr_pool,
        ):
            psum_col = pcol_pool.tile([OHW, W], F32)
            oh = consts.tile([128, BP * OHW], BF16)
            ones_oh = consts.tile([128, BP * OHW], BF16)
            any_t = small.tile([128, 4 * BP], F32)
            rvals = small.tile([128, 2 * BP], F32)

            neg_half = consts.tile([128, 1], F32)
            hm_lo_i = consts.tile([128, GROUP * 4], I32)
            hm_lo = consts.tile([128, GROUP * 4], F32)
            hm_hi_i = consts.tile([128, GROUP * 4], I32)
            hm_hi = consts.tile([128, GROUP * 4], F32)
            wm_lo_i = consts.tile([BP, W], I32)
            wm_lo = consts.tile([BP, W], BF16)
            wm_hi_i = consts.tile([BP, W], I32)
            wm_hi = consts.tile([BP, W], BF16)
            ones128 = consts.tile([128, 128], F32)
            ident = consts.tile([128, 128], F32)
            dflt = consts.tile([BP, 4], F32)

            def emit_early_consts():
                nc.gpsimd.memset(neg_half[:], -0.5)
                # OH[p, i*OHW + i] = 1: routes image i to PSUM row i
                nc.gpsimd.memset(ones_oh[:], 1.0)
                nc.gpsimd.affine_select(
                    oh[:], ones_oh[:], [[-1, BP], [1, OHW]], OP.is_equal, 0.0,
                    base=0, channel_multiplier=0,
                )
                # row-index tables: y = 4p + r; lo = y - 512, hi = y + 1
                nc.gpsimd.iota(hm_lo_i[:], [[0, GROUP], [1, 4]], base=-512,
                               channel_multiplier=4)
                nc.gpsimd.tensor_copy(hm_lo[:], hm_lo_i[:])
                nc.gpsimd.iota(hm_hi_i[:], [[0, GROUP], [1, 4]], base=1,
                               channel_multiplier=4)
                nc.gpsimd.tensor_copy(hm_hi[:], hm_hi_i[:])

            def emit_tail_consts():
                # col-index tables: lo = j - 512, hi = j + 1
                nc.gpsimd.iota(wm_lo_i[:], [[1, W]], base=-512, channel_multiplier=0)
                nc.gpsimd.tensor_copy(wm_lo[:], wm_lo_i[:])
                nc.gpsimd.iota(wm_hi_i[:], [[1, W]], base=1, channel_multiplier=0)
                nc.gpsimd.tensor_copy(wm_hi[:], wm_hi_i[:])
                nc.gpsimd.memset(ones128[:], 1.0)
                nc.gpsimd.affine_select(
                    ident[:], ones128[:], [[-1, 128]], OP.is_equal, 0.0,
                    base=0, channel_multiplier=1,
                )
                nc.gpsimd.memset(dflt[:, 0:2], 0.25)
                nc.gpsimd.memset(dflt[:, 2:4], 0.75)

            # row-side groups; last group is a single image
            row_groups = [(0, 8), (8, 8), (16, 8), (24, 7), (31, 1)]

            def finish_group(start, n):
                cs = slice(4 * start, 4 * (start + n))
                rt_lo = small.tile([128, 4 * GROUP], F32, tag="rt_lo")
                nc.vector.scalar_tensor_tensor(
                    rt_lo[:, 0:4 * n], any_t[:, cs], ANY_THR, hm_lo[:, 0:4 * n],
                    OP.is_gt, OP.mult)
                nc.vector.tensor_reduce(
                    rvals[:, start:start + n],
                    rt_lo[:, 0:4 * n].rearrange("p (i r) -> p i r", r=4),
                    op=OP.min, axis=AX.X)
                rt_hi = small.tile([128, 4 * GROUP], F32, tag="rt_hi")
                nc.vector.scalar_tensor_tensor(
                    rt_hi[:, 0:4 * n], any_t[:, cs], ANY_THR, hm_hi[:, 0:4 * n],
                    OP.is_gt, OP.mult)
                nc.vector.tensor_reduce(
                    rvals[:, BP + start:BP + start + n],
                    rt_hi[:, 0:4 * n].rearrange("p (i r) -> p i r", r=4),
                    op=OP.max, axis=AX.X)

            # per-image row-block engine split: 'd' = DVE is_gt {0,1} count,
            # 'a' = ACT Relu(x-0.5) sum. For the last two images DVE takes the
            # late row blocks (it runs the rest of the tail anyway).
            def row_engines(i):
                if i >= BP - 2:
                    return "aadd"
                return "ddaa"

            # first image starts with a small piece so the stream begins
            # sooner; last two stream per row-block so tail threshold/matmul
            # latency tracks the last arriving bytes (HWDGE: pieces cost no
            # Q7 time)
            def dma_pieces(i):
                if i == 0:
                    return [(0, 1), (1, 4)]
                if i >= BP - 2:
                    return [(0, 1), (1, 2), (2, 3), (3, 4)]
                return None  # one unsliced full-image DMA

            def emit_dma(i):
                img = imgs.tile([128, IMG_FREE], F32, tag="img")
                pieces = dma_pieces(i)
                if pieces is None:
                    # CRITICAL: unsliced APs. Even a full-range a-dim slice
                    # (xv[:, i:i+1, 0:4]) defeats descriptor coalescing -
                    # per-partition runs split 8 KiB -> 4 x 2 KiB and every
                    # DMA engine runs ~12% slower (traced v9/v10).
                    nc.sync.dma_start(
                        out=img[:].rearrange("p (a w) -> p a w", a=4),
                        in_=xv[:, i:i + 1],
                    )
                else:
                    img3 = img[:].rearrange("p (a w) -> p a w", a=4)
                    for (al, ah) in pieces:
                        nc.sync.dma_start(
                            out=img3[:, al:ah, :],
                            in_=xv[:, i:i + 1, al:ah],
                        )
                return img

            def emit_compute(i, img):
                m01 = masks.tile([128, IMG_FREE], BF16, tag="m01")
                for r, eng in enumerate(row_engines(i)):
                    sl = slice(r * W, (r + 1) * W)
                    acc = any_t[:, 4 * i + r:4 * i + r + 1]
                    if eng == "d":
                        nc.vector.tensor_scalar(
                            m01[:, sl], img[:, sl], 0.5, None,
                            OP.is_gt, OP.add, accum_out=acc)
                    else:
                        nc.scalar.activation(
                            m01[:, sl], img[:, sl], AF.Relu,
                            bias=neg_half[:], accum_out=acc)
                for r in range(4):
                    sl = slice(r * W, (r + 1) * W)
                    nc.tensor.matmul(
                        psum_col[:, :], oh[:, i * OHW:i * OHW + OHW], m01[:, sl],
                        start=(i == 0 and r == 0), stop=(i == BP - 1 and r == 3),
                    )
                for (gs, gn) in row_groups:
                    if gs + gn - 1 == i:
                        finish_group(gs, gn)

            emit_early_consts()
            emit_tail_consts()
            for i in range(BP):
                img = emit_dma(i)
                emit_compute(i, img)

            # ---- tail ----
            braw = small.tile([BP, 4], F32)

            # col side: one (colsum > thr) any-mask off PSUM (f32 read), then
            # bf16 iota products + reduces (2x DVE rate; bf16 iota rounding is
            # <= 2 columns = 0.4% of W, well inside tolerance)
            colany = small.tile([BP, W], BF16)
            nc.vector.tensor_scalar(
                colany[:], psum_col[0:BP, :], ANY_THR, None, OP.is_gt)
            scr_lo = small.tile([BP, W], BF16)
            scr_hi = small.tile([BP, W], BF16)
            nc.vector.tensor_tensor(scr_lo[:], colany[:], wm_lo[:], op=OP.mult)
            nc.vector.tensor_reduce(braw[:, 0:1], scr_lo[:], op=OP.min, axis=AX.X)
            nc.vector.tensor_tensor(scr_hi[:], colany[:], wm_hi[:], op=OP.mult)
            nc.vector.tensor_reduce(braw[:, 2:3], scr_hi[:], op=OP.max, axis=AX.X)

            # row side: transpose rvals, reduce per image
            rT = ptr_pool.tile([2 * BP, 128], F32)
            nc.tensor.transpose(rT[:], rvals[:], ident[:])
            nc.vector.tensor_reduce(braw[:, 1:2], rT[0:BP, :], op=OP.min, axis=AX.X)
            nc.vector.tensor_reduce(braw[:, 3:4], rT[BP:2 * BP, :], op=OP.max, axis=AX.X)

            # empty mask: y_max raw is 0 iff no foreground
            emp = small.tile([BP, 1], F32)
            nc.vector.tensor_scalar(emp[:], braw[:, 3:4], 0.5, None, OP.is_lt)

            # normalize: lo = (v + 512)/512, hi = (v - 1)/512
            boxes = small.tile([BP, 4], F32)
            nc.vector.tensor_scalar(
                boxes[:, 0:2], braw[:, 0:2], 512.0, 1.0 / 512, OP.add, OP.mult)
            nc.vector.tensor_scalar(
                boxes[:, 2:4], braw[:, 2:4], 1.0, 1.0 / 512, OP.subtract, OP.mult)

            # expand-small fused: lo' = min(lo, max(0, c - s/2)),
            #                     hi' = max(hi, min(1, c + s/2))
            csum = small.tile([BP, 2], F32)
            lo2 = small.tile([BP, 2], F32)
            hi2 = small.tile([BP, 2], F32)
            final = small.tile([BP, 4], F32)
            nc.vector.tensor_add(csum[:], boxes[:, 0:2], boxes[:, 2:4])
            nc.vector.tensor_scalar(
                lo2[:], csum[:], 0.5, MIN_BOX * 0.5, OP.mult, OP.subtract)
            nc.vector.tensor_scalar(lo2[:], lo2[:], 0.0, None, OP.max)
            nc.vector.tensor_tensor(final[:, 0:2], boxes[:, 0:2], lo2[:], op=OP.min)
            nc.vector.tensor_scalar(
                hi2[:], csum[:], 0.5, MIN_BOX * 0.5, OP.mult, OP.add)
            nc.vector.tensor_scalar(hi2[:], hi2[:], 1.0, None, OP.min)
            nc.vector.tensor_tensor(final[:, 2:4], boxes[:, 2:4], hi2[:], op=OP.max)

            # default box where empty: final += (default - final) * emp
            dmb = small.tile([BP, 4], F32)
            nc.vector.tensor_sub(dmb[:], dflt[:], final[:])
            outb = small.tile([BP, 4], F32)
            nc.vector.scalar_tensor_tensor(
                outb[:], dmb[:], emp[:], final[:], OP.mult, OP.add)

            nc.sync.dma_start(out=out.ap(), in_=outb[:])

    return nc


_NC = None


def _get_nc():
    global _NC
    if _NC is None:
        nc = build_nc()
        nc.compile()
        _NC = nc
    return _NC


def kernel(mask_fg: np.ndarray) -> np.ndarray:
    mask_fg = np.ascontiguousarray(np.asarray(mask_fg, dtype=np.float32))
    assert mask_fg.shape == (B, 1, H, W), mask_fg.shape
    nc = _get_nc()
    shards = mask_fg.reshape(N_CORES, BP, 1, H, W)
    in_maps = [{"mask_fg": np.ascontiguousarray(shards[i])} for i in range(N_CORES)]
    res = run_bass_kernel_spmd(nc, in_maps, core_ids=list(range(N_CORES)))
    return np.concatenate(
        [res.results[i]["out"] for i in range(N_CORES)], axis=0
    ).astype(np.float32)


# revision 25
# speedup vs baseline: 1.1504x; 1.0020x over previous
"""Trainium2 Bass kernel for nn_BBoxGenerator (segment_reduce).

mask_fg (256, 1, 512, 512) f32 -> boxes (256, 4) f32 [x0, y0, x1, y1].

Pure data parallel: each of the 8 cores handles 32 images independently.

v8: HWDGE (nc.sync) f32 stream + reworked compute/finishing.
  Why HWDGE: SWDGE descriptor generation runs on GpSimd's Q7 and its
  descriptor rings live in SBUF; DVE tensor_scalar ops (our per-image
  threshold) enter 2-port perf mode which blocks GpSimd, and the ring
  fetches contend on the AXI ports serving SDMA engines 7/15 - traced
  as a sustained +16% per-descriptor slowdown on DMA_15 that paced the
  whole stream (v3-v7, 120-126us). HWDGE has no SBUF rings and never
  contends with DVE; the cost is losing the f32->bf16 cast-during-DMA,
  so images land as f32 (16 x 1 MiB SBUF buffers) and the threshold
  ops read f32.
  - Unified "positive iff any" masks: DVE rows use (x > 0.5) {0,1} with
    fused accum row sums; ACT rows use Relu(x-0.5) >= 0 with fused accum
    row sums. Row sums and PSUM column sums are all "> 0.002 iff any
    foreground": one threshold everywhere, PSUM rows 0..31 in image
    order, ONE output DMA (no un-permute).
  - Row side: per-group masked-iota min/max into rvals during the
    stream; one PE transpose + two small reduces at the tail.
  - Col side: fused (colsum > thr) * iota via scalar_tensor_tensor +
    tensor_reduce min/max. (tensor_tensor_reduce would fuse further but
    crashes the HW: NRT_EXEC_UNIT_UNRECOVERABLE, micro_test.py stage 5.)
  - Box expand fused without predicates:
    lo' = min(lo, max(0, c - s/2)), hi' = max(hi, min(1, c + s/2)).
  - Q7 (gpsimd) paces SWDGE descriptor emission ring-full, so const
    emission is split: tiny block (one-hot + row tables) right after
    dma1 while the ring is shallow; tail-only consts before dma30 where
    the deep ring absorbs the pause.
"""

import numpy as np

from concourse import bacc, mybir
from concourse.tile import TileContext
from concourse.bass_utils import run_bass_kernel_spmd

F32 = mybir.dt.float32
BF16 = mybir.dt.bfloat16
I32 = mybir.dt.int32
OP = mybir.AluOpType
AX = mybir.AxisListType
AF = mybir.ActivationFunctionType

N_CORES = 8
B = 256
BP = B // N_CORES  # 32 images per core
H = W = 512
IMG_FREE = 4 * W  # 2048 free elems per image (4 rows per partition)
GROUP = 8
OHW = 64  # one-hot block width per image (v1 width; rows 32..63 unused)

MIN_BOX = 0.05
ANY_THR = 0.002  # any_t/PSUM sums exceed this iff any foreground


def build_nc():
    nc = bacc.Bacc("TRN2", target_bir_lowering=False, debug=False, num_devices=N_CORES)
    x = nc.declare_dram_parameter("mask_fg", [BP, 1, H, W], F32, isOutput=False)
    out = nc.declare_dram_parameter("out", [BP, 4], F32, isOutput=True)

    # (128, BP, 4, 512): partition p holds rows 4p..4p+3 of each image
    xv = x.ap().rearrange("b one (p a) w -> p (b one) a w", p=128)

    with TileContext(nc) as tc:
        with (
            tc.tile_pool(name="imgs", bufs=16) as imgs,
            tc.tile_pool(name="masks", bufs=6) as masks,
            tc.tile_pool(name="small", bufs=1) as small,
            tc.tile_pool(name="pcol", bufs=1, space="PSUM") as pcol_pool,
            tc.tile_pool(name="ptr", bufs=1, space="PSUM") as ptr_pool,
        ):
            psum_col = pcol_pool.tile([OHW, W], F32)
            oh = small.tile([128, BP * OHW], BF16)
            ones_oh = small.tile([128, BP * OHW], BF16)
            any_t = small.tile([128, 4 * BP], F32)
            rvals = small.tile([128, 2 * BP], F32)

            neg_half = small.tile([128, 1], F32)
            hm_lo_i = small.tile([128, GROUP * 4], I32)
            hm_lo = small.tile([128, GROUP * 4], F32)
            hm_hi_i = small.tile([128, GROUP * 4], I32)
            hm_hi = small.tile([128, GROUP * 4], F32)
            wm_lo_i = small.tile([BP, W], I32)
            wm_lo = small.tile([BP, W], BF16)
            wm_hi_i = small.tile([BP, W], I32)
            wm_hi = small.tile([BP, W], BF16)
            ones128 = small.tile([128, 128], F32)
            ident = small.tile([128, 128], F32)
            dflt = small.tile([BP, 4], F32)

            def emit_early_consts():
                nc.gpsimd.memset(neg_half[:], -0.5)
                # OH[p, i*OHW + i] = 1: routes image i to PSUM row i
                nc.gpsimd.memset(ones_oh[:], 1.0)
                nc.gpsimd.affine_select(
                    oh[:], ones_oh[:], [[-1, BP], [1, OHW]], OP.is_equal, 0.0,
                    base=0, channel_multiplier=0,
                )
                # row-index tables: y = 4p + r; lo = y - 512, hi = y + 1
                nc.gpsimd.iota(hm_lo_i[:], [[0, GROUP], [1, 4]], base=-512,
                               channel_multiplier=4)
                nc.gpsimd.tensor_copy(hm_lo[:], hm_lo_i[:])
                nc.gpsimd.iota(hm_hi_i[:], [[0, GROUP], [1, 4]], base=1,
                               channel_multiplier=4)
                nc.gpsimd.tensor_copy(hm_hi[:], hm_hi_i[:])

            def emit_tail_consts():
                # col-index tables: lo = j - 512, hi = j + 1
                nc.gpsimd.iota(wm_lo_i[:], [[1, W]], base=-512, channel_multiplier=0)
                nc.gpsimd.tensor_copy(wm_lo[:], wm_lo_i[:])
                nc.gpsimd.iota(wm_hi_i[:], [[1, W]], base=1, channel_multiplier=0)
                nc.gpsimd.tensor_copy(wm_hi[:], wm_hi_i[:])
                nc.gpsimd.memset(ones128[:], 1.0)
                nc.gpsimd.affine_select(
                    ident[:], ones128[:], [[-1, 128]], OP.is_equal, 0.0,
                    base=0, channel_multiplier=1,
                )
                nc.gpsimd.memset(dflt[:, 0:2], 0.25)
                nc.gpsimd.memset(dflt[:, 2:4], 0.75)

            # row-side groups; last group is a single image
            row_groups = [(0, 8), (8, 8), (16, 8), (24, 6), (30, 1), (31, 1)]

            def finish_group(start, n):
                cs = slice(4 * start, 4 * (start + n))
                rt_lo = small.tile([128, 4 * GROUP], F32, tag="rt_lo")
                nc.vector.scalar_tensor_tensor(
                    rt_lo[:, 0:4 * n], any_t[:, cs], ANY_THR, hm_lo[:, 0:4 * n],
                    OP.is_gt, OP.mult)
                nc.vector.tensor_reduce(
                    rvals[:, start:start + n],
                    rt_lo[:, 0:4 * n].rearrange("p (i r) -> p i r", r=4),
                    op=OP.min, axis=AX.X)
                rt_hi = small.tile([128, 4 * GROUP], F32, tag="rt_hi")
                nc.vector.scalar_tensor_tensor(
                    rt_hi[:, 0:4 * n], any_t[:, cs], ANY_THR, hm_hi[:, 0:4 * n],
                    OP.is_gt, OP.mult)
                nc.vector.tensor_reduce(
                    rvals[:, BP + start:BP + start + n],
                    rt_hi[:, 0:4 * n].rearrange("p (i r) -> p i r", r=4),
                    op=OP.max, axis=AX.X)

            # per-image row-block engine split: 'd' = DVE is_gt {0,1} count,
            # 'a' = ACT Relu(x-0.5) sum. For the last two images DVE takes the
            # late row blocks (it runs the rest of the tail anyway).
            def row_engines(i):
                if i >= BP - 2:
                    return "aadd"
                return "ddaa"

            # first image starts with a small piece so the stream begins
            # sooner; last two stream per row-block so tail threshold/matmul
            # latency tracks the last arriving bytes (HWDGE: pieces cost no
            # Q7 time)
            def dma_pieces(i):
                if i == 0:
                    return [(0, 1), (1, 4)]
                if i >= BP - 2:
                    return [(0, 1), (1, 2), (2, 3), (3, 4)]
                return None  # one unsliced full-image DMA

            def emit_dma_pair(i):
                # one unsliced 2 MiB DMA covering images i, i+1: per-partition
                # runs stay 8 KiB-coalesced, halves DMA completions
                img2 = imgs.tile([128, 2 * IMG_FREE], F32, tag="imgpair", bufs=6)
                nc.sync.dma_start(
                    out=img2[:].rearrange("p (b a w) -> p b a w", b=2, a=4),
                    in_=xv[:, i:i + 2],
                )
                return img2[:, 0:IMG_FREE], img2[:, IMG_FREE:2 * IMG_FREE]

            def emit_dma(i):
                img = imgs.tile([128, IMG_FREE], F32, tag="img", bufs=6)
                pieces = dma_pieces(i)
                if pieces is None:
                    # CRITICAL: unsliced APs. Even a full-range a-dim slice
                    # (xv[:, i:i+1, 0:4]) defeats descriptor coalescing -
                    # per-partition runs split 8 KiB -> 4 x 2 KiB and every
                    # DMA engine runs ~12% slower (traced v9/v10).
                    nc.sync.dma_start(
                        out=img[:].rearrange("p (a w) -> p a w", a=4),
                        in_=xv[:, i:i + 1],
                    )
                else:
                    img3 = img[:].rearrange("p (a w) -> p a w", a=4)
                    for (al, ah) in pieces:
                        nc.sync.dma_start(
                            out=img3[:, al:ah, :],
                            in_=xv[:, i:i + 1, al:ah],
                        )
                return img

            def emit_compute(i, img):
                m01 = masks.tile([128, IMG_FREE], BF16, tag="m01")
                for r, eng in enumerate(row_engines(i)):
                    sl = slice(r * W, (r + 1) * W)
                    acc = any_t[:, 4 * i + r:4 * i + r + 1]
                    if eng == "d":
                        nc.vector.tensor_scalar(
                            m01[:, sl], img[:, sl], 0.5, None,
                            OP.is_gt, OP.add, accum_out=acc)
                    else:
                        nc.scalar.activation(
                            m01[:, sl], img[:, sl], AF.Relu,
                            bias=neg_half[:], accum_out=acc)
                for r in range(4):
                    sl = slice(r * W, (r + 1) * W)
                    nc.tensor.matmul(
                        psum_col[:, :], oh[:, i * OHW:i * OHW + OHW], m01[:, sl],
                        start=(i == 0 and r == 0), stop=(i == BP - 1 and r == 3),
                    )
                for (gs, gn) in row_groups:
                    if gs + gn - 1 == i:
                        finish_group(gs, gn)

            emit_early_consts()
            emit_tail_consts()
            i = 0
            while i < BP:
                if 1 <= i <= 25:
                    a, b = emit_dma_pair(i)
                    emit_compute(i, a)
                    emit_compute(i + 1, b)
                    i += 2
                else:
                    img = emit_dma(i)
                    emit_compute(i, img)
                    i += 1

            # ---- tail ----
            braw = small.tile([BP, 4], F32)

            # col side: one (colsum > thr) any-mask off PSUM (f32 read), then
            # bf16 iota products + reduces (2x DVE rate; bf16 iota rounding is
            # <= 2 columns = 0.4% of W, well inside tolerance)
            colany = small.tile([BP, W], BF16)
            nc.vector.tensor_scalar(
                colany[:], psum_col[0:BP, :], ANY_THR, None, OP.is_gt)
            scr_lo = small.tile([BP, W], BF16)
            scr_hi = small.tile([BP, W], BF16)
            nc.vector.tensor_tensor(scr_lo[:], colany[:], wm_lo[:], op=OP.mult)
            nc.vector.tensor_reduce(braw[:, 0:1], scr_lo[:], op=OP.min, axis=AX.X)
            nc.vector.tensor_tensor(scr_hi[:], colany[:], wm_hi[:], op=OP.mult)
            nc.vector.tensor_reduce(braw[:, 2:3], scr_hi[:], op=OP.max, axis=AX.X)

            # row side: transpose rvals, reduce per image
            rT = ptr_pool.tile([2 * BP, 128], F32)
            nc.tensor.transpose(rT[:], rvals[:], ident[:])
            nc.vector.tensor_reduce(braw[:, 1:2], rT[0:BP, :], op=OP.min, axis=AX.X)
            nc.vector.tensor_reduce(braw[:, 3:4], rT[BP:2 * BP, :], op=OP.max, axis=AX.X)

            # empty mask: y_max raw is 0 iff no foreground
            emp = small.tile([BP, 1], F32)
            nc.vector.tensor_scalar(emp[:], braw[:, 3:4], 0.5, None, OP.is_lt)

            # normalize: lo = (v + 512)/512, hi = (v - 1)/512
            boxes = small.tile([BP, 4], F32)
            nc.vector.tensor_scalar(
                boxes[:, 0:2], braw[:, 0:2], 512.0, 1.0 / 512, OP.add, OP.mult)
            nc.vector.tensor_scalar(
                boxes[:, 2:4], braw[:, 2:4], 1.0, 1.0 / 512, OP.subtract, OP.mult)

            # expand-small fused: lo' = min(lo, max(0, c - s/2)),
            #                     hi' = max(hi, min(1, c + s/2))
            csum = small.tile([BP, 2], F32)
            lo2 = small.tile([BP, 2], F32)
            hi2 = small.tile([BP, 2], F32)
            final = small.tile([BP, 4], F32)
            nc.vector.tensor_add(csum[:], boxes[:, 0:2], boxes[:, 2:4])
            nc.vector.tensor_scalar(
                lo2[:], csum[:], 0.5, MIN_BOX * 0.5, OP.mult, OP.subtract)
            nc.vector.scalar_tensor_tensor(
                final[:, 0:2], lo2[:], 0.0, boxes[:, 0:2], OP.max, OP.min)
            nc.vector.tensor_scalar(
                hi2[:], csum[:], 0.5, MIN_BOX * 0.5, OP.mult, OP.add)
            nc.vector.scalar_tensor_tensor(
                final[:, 2:4], hi2[:], 1.0, boxes[:, 2:4], OP.min, OP.max)

            # default box where empty: final += (default - final) * emp
            dmb = small.tile([BP, 4], F32)
            nc.vector.tensor_sub(dmb[:], dflt[:], final[:])
            outb = small.tile([BP, 4], F32)
            nc.vector.scalar_tensor_tensor(
                outb[:], dmb[:], emp[:], final[:], OP.mult, OP.add)

            nc.sync.dma_start(out=out.ap(), in_=outb[:])

    return nc


_NC = None


def _get_nc():
    global _NC
    if _NC is None:
        nc = build_nc()
        nc.compile()
        _NC = nc
    return _NC


def kernel(mask_fg: np.ndarray) -> np.ndarray:
    mask_fg = np.ascontiguousarray(np.asarray(mask_fg, dtype=np.float32))
    assert mask_fg.shape == (B, 1, H, W), mask_fg.shape
    nc = _get_nc()
    shards = mask_fg.reshape(N_CORES, BP, 1, H, W)
    in_maps = [{"mask_fg": np.ascontiguousarray(shards[i])} for i in range(N_CORES)]
    res = run_bass_kernel_spmd(nc, in_maps, core_ids=list(range(N_CORES)))
    return np.concatenate(
        [res.results[i]["out"] for i in range(N_CORES)], axis=0
    ).astype(np.float32)
